# revision 1
# baseline (speedup 1.0000x reference)
"""Single-head memory attention on Trainium2, batch-parallel across 8 NeuronCores.

Per core (one batch element):
    Q^T = Wq @ x^T + bq                  (MM1, bf16, fp32 accum)
    S^T = keys @ Q^T                     (MM2; k on partitions, q on free dim)
    E^T = exp(S^T/sqrt(d) + mask_k)      (one ScalarE activation: scale+bias+exp)
    sums= ones^T @ E^T                   (N=512 matmuls into a [1,512] PSUM row)
    O   = E^T.T @ V  * recip(sums)       (MM3 + per-partition normalize)

Operand transposes (x^T, keys^T, Wq^T) are 128x128 TensorE transposes.
(The DMA-xbar transpose path is faster on paper but Tile must globally
serialize all DMA around every transpose-mode transfer — a known HW-hang
workaround — which starves the whole pipeline.)
"""

import numpy as np

import concourse.bacc as bacc
import concourse.mybir as mybir
from concourse.tile import TileContext
from concourse.masks import make_identity
from concourse.bass_utils import run_bass_kernel_spmd

B, LQ, LK, D = 8, 2048, 2048, 1024
P = 128
QCH = 512                 # queries processed per chunk
NQC = LQ // QCH           # 4 chunks
NDT = D // P              # 8 tiles along d (contraction of MM1)
NET = D // P              # 8 tiles along e (contraction of MM2)
NKT = LK // P             # 16 tiles along k (contraction of MM3)
NQS = QCH // P            # 4 query subtiles per chunk
SCALE = 1.0 / float(np.sqrt(D))

F32 = mybir.dt.float32
BF16 = mybir.dt.bfloat16
AFT = mybir.ActivationFunctionType

_CACHE = {}


def build_nc():
    nc = bacc.Bacc(None, target_bir_lowering=False)

    x_d = nc.dram_tensor("x", [LQ, D], F32, kind="ExternalInput")
    keys_d = nc.dram_tensor("keys", [LK, D], F32, kind="ExternalInput")
    values_d = nc.dram_tensor("values", [LK, D], F32, kind="ExternalInput")
    mask_d = nc.dram_tensor("mask", [LK, 1], F32, kind="ExternalInput")
    wq_d = nc.dram_tensor("Wq", [D, D], F32, kind="ExternalInput")
    bq_d = nc.dram_tensor("bq", [D], F32, kind="ExternalInput")
    out_d = nc.dram_tensor("out", [LQ, D], F32, kind="ExternalOutput")

    with TileContext(nc) as tc:
        with (
            tc.tile_pool(name="persist", bufs=1) as persist,
            tc.tile_pool(name="stage", bufs=8) as stagep,
            tc.tile_pool(name="cvt", bufs=4) as cvtp,
            tc.tile_pool(name="xTp", bufs=2) as xTp,
            tc.tile_pool(name="QTp", bufs=2) as QTp,
            tc.tile_pool(name="ETp", bufs=2) as ETp,
            tc.tile_pool(name="osb", bufs=3) as osbp,
            tc.tile_pool(name="sums", bufs=2) as sumsp,
            tc.tile_pool(name="psT", bufs=2, space="PSUM") as psTp,
            tc.tile_pool(name="psAcc", bufs=5, space="PSUM") as psAccp,
            tc.tile_pool(name="psD", bufs=1, space="PSUM") as psDp,
            tc.tile_pool(name="dram", bufs=2, space="DRAM") as dramp,
        ):
            # ---- constants ----
            ident = persist.tile([P, P], BF16)
            make_identity(nc, ident)
            ones = persist.tile([P, 1], BF16)
            nc.any.memset(ones, 1.0)
            bq_sb = persist.tile([P, NDT], F32)
            mask_sb = persist.tile([P, NKT], F32)

            # ---- persistent operands ----
            WqT = persist.tile([P, NDT, D], BF16)    # [d%P, d//P, e] = Wq[e, d]
            keysT = persist.tile([P, NET, LK], BF16)  # [e%P, e//P, k] = keys[k, e]
            Vsb = persist.tile([P, NKT, D], BF16)    # [k%P, k//P, dv] = values[k, dv]

            copy_eng = [
                lambda o, i: nc.vector.tensor_copy(o, i),
                lambda o, i: nc.scalar.copy(o, i),
            ]
            state = {"n": 0}

            def transpose_block(dst3, col0, cv):
                # dst3[:, ft, col0:col0+P] = cv[:, ft*P:(ft+1)*P].T for ft in 0..7
                # All eight 128x128 transposes of one staged tile fill exactly
                # one 2KB PSUM bank, drained by a single strided copy.
                pt = psTp.tile([P, NDT, P], BF16, tag="pst")
                for ft in range(NDT):
                    nc.tensor.transpose(
                        pt[:, ft, :], cv[:, ft * P:(ft + 1) * P], ident
                    )
                copy_eng[state["n"] % 2](dst3[:, :, col0:col0 + P], pt)
                state["n"] += 1

            def stage_rows(dram_rows, parity):
                st = stagep.tile([P, D], F32, tag="stage")
                nc.sync.dma_start(st, dram_rows)
                cv = cvtp.tile([P, D], BF16, tag="cvt")
                cvt = nc.vector.tensor_copy if parity % 2 == 0 else nc.scalar.copy
                cvt(cv, st)
                return cv

            # x chunk staging: xT[p, dt, q'] = x[qc*QCH+q', dt*P+p]
            def x_stage(qc):
                xT = xTp.tile([P, NDT, QCH], BF16, tag="xT")
                for qs in range(NQS):
                    r0 = qc * QCH + qs * P
                    cv = stage_rows(x_d[r0:r0 + P, :], qs)
                    transpose_block(xT, qs * P, cv)
                return xT

            def mm1(xT):
                # QT[e, q] = Wq @ x^T + bq
                QT = QTp.tile([P, NET, QCH], BF16, tag="QT")
                for et in range(NET):
                    pq = psAccp.tile([P, QCH], F32, tag="acc")
                    for dt in range(NDT):
                        nc.tensor.matmul(
                            pq,
                            WqT[:, dt, et * P:(et + 1) * P],
                            xT[:, dt, :],
                            start=(dt == 0),
                            stop=(dt == NDT - 1),
                        )
                    nc.vector.tensor_scalar_add(QT[:, et, :], pq, bq_sb[:, et:et + 1])
                return QT

            def mm2_phase(QT):
                # MM2 + exp + denominator accumulation
                ET = ETp.tile([P, NKT, QCH], BF16, tag="ET")
                pd = psDp.tile([1, QCH], F32, tag="psd")
                for kt in range(NKT):
                    ps = psAccp.tile([P, QCH], F32, tag="acc")
                    for et in range(NET):
                        nc.tensor.matmul(
                            ps,
                            keysT[:, et, kt * P:(kt + 1) * P],
                            QT[:, et, :],
                            start=(et == 0),
                            stop=(et == NET - 1),
                        )
                    nc.scalar.activation(
                        ET[:, kt, :], ps, AFT.Exp,
                        bias=mask_sb[:, kt:kt + 1], scale=SCALE,
                    )
                # denominator matmuls batched back-to-back: the ones column
                # stays loaded, and no accumulation-group breaks mid-MM2
                for kt in range(NKT):
                    nc.tensor.matmul(
                        pd, ones, ET[:, kt, :],
                        start=(kt == 0), stop=(kt == NKT - 1),
                    )

                # denominator: [1, 512] -PSUM-> SBUF -> DRAM -> [128, 4] -> recip
                # (an SBUF->SBUF partition-scatter AP silently corrupts on HW,
                # so bounce the 2KB row through a DRAM scratch tile)
                sums_sb = sumsp.tile([1, QCH], F32, tag="sums")
                nc.vector.tensor_copy(sums_sb, pd)
                scr = dramp.tile([1, QCH], F32, tag="scr")
                nc.sync.dma_start(scr[:, :], sums_sb)
                sums_t = sumsp.tile([P, NQS], F32, tag="sumst")
                nc.sync.dma_start(
                    sums_t, scr[:, :].rearrange("o (a p) -> p (o a)", p=P)
                )
                rc = sumsp.tile([P, NQS], F32, tag="rc")
                nc.vector.reciprocal(rc, sums_t)
                return ET, rc

            def mm3_phase(qc, ET, rc):
                # MM3: O[q, dv] = sum_k E[k,q] V[k,dv], then normalize
                # (drains alternate DVE / ACT so neither engine's queue
                # becomes the psO-recycling bottleneck)
                for qs in range(NQS):
                    osb = osbp.tile([P, D], F32, tag="osb")
                    for dv in range(2):
                        po = psAccp.tile([P, QCH], F32, tag="acc")
                        for kt in range(NKT):
                            nc.tensor.matmul(
                                po,
                                ET[:, kt, qs * P:(qs + 1) * P],
                                Vsb[:, kt, dv * QCH:(dv + 1) * QCH],
                                start=(kt == 0),
                                stop=(kt == NKT - 1),
                            )
                        oslice = osb[:, dv * QCH:(dv + 1) * QCH]
                        if (qs * 2 + dv) % 2 == 0:
                            nc.vector.tensor_scalar_mul(oslice, po, rc[:, qs:qs + 1])
                        else:
                            nc.scalar.activation(
                                oslice, po, AFT.Copy, bias=0.0,
                                scale=rc[:, qs:qs + 1],
                            )
                        nc.sync.dma_start(
                            out_d[qc * QCH + qs * P: qc * QCH + (qs + 1) * P,
                                  dv * QCH:(dv + 1) * QCH],
                            oslice,
                        )

            # ---- emission ----
            # Wq -> WqT
            for et in range(D // P):
                cv = stage_rows(wq_d[et * P:(et + 1) * P, :], et)
                transpose_block(WqT, et * P, cv)
            xT0 = x_stage(0)
            # small const loads after the bulk staging is queued
            nc.sync.dma_start(bq_sb, bq_d[:].rearrange("(t p) -> p t", p=P))
            nc.sync.dma_start(mask_sb, mask_d[:].rearrange("(t p) o -> p (t o)", p=P))
            # MM1 first: 14us of dense PE work covers the keys DMA window,
            # so the keys transposes that follow never wait on staging
            QT0 = mm1(xT0)
            # keys -> keysT
            for kt in range(NKT):
                cv = stage_rows(keys_d[kt * P:(kt + 1) * P, :], kt)
                transpose_block(keysT, kt * P, cv)
            # values -> Vsb (no transpose)
            for kt in range(NKT):
                st = stagep.tile([P, D], F32, tag="stage")
                nc.sync.dma_start(st, values_d[kt * P:(kt + 1) * P, :])
                cvt = nc.vector.tensor_copy if kt % 2 == 0 else nc.scalar.copy
                cvt(Vsb[:, kt, :], st)
            ET, rc = mm2_phase(QT0)
            xT_next = x_stage(1)
            mm3_phase(0, ET, rc)
            for qc in range(1, NQC):
                QT = mm1(xT_next)
                ET, rc = mm2_phase(QT)
                if qc + 1 < NQC:
                    xT_next = x_stage(qc + 1)
                mm3_phase(qc, ET, rc)

    nc.finalize()
    return nc


def _get_nc():
    if "nc" not in _CACHE:
        _CACHE["nc"] = build_nc()
    return _CACHE["nc"]


def kernel(x, mem_padding_mask, keys, values, Wq, bq):
    nc = _get_nc()
    Wq_c = np.ascontiguousarray(Wq, dtype=np.float32)
    bq_c = np.ascontiguousarray(bq, dtype=np.float32)
    in_maps = [
        {
            "x": np.ascontiguousarray(x[b], dtype=np.float32),
            "keys": np.ascontiguousarray(keys[b], dtype=np.float32),
            "values": np.ascontiguousarray(values[b], dtype=np.float32),
            "mask": np.ascontiguousarray(mem_padding_mask[b], dtype=np.float32),
            "Wq": Wq_c,
            "bq": bq_c,
        }
        for b in range(B)
    ]
    res = run_bass_kernel_spmd(nc, in_maps, core_ids=list(range(B)))
    return np.stack([res.results[i]["out"] for i in range(B)], axis=0).astype(np.float32)

